# revision 1
# baseline (speedup 1.0000x reference)
"""Trainium2 Bass kernel for DCKModule (involution / dynamic conv kernel).

Math (per batch image, all fp32):
  x  = relu(W1 @ guide * bn_scale + bn_bias)        # (64, 9216)
  df = W2 @ x                                       # (784, 9216) = (16 groups * 49 taps)
  out[g,gc,p] = sum_k df[g,k,p] * fpad[g,gc, p+off_k] + feature[g,gc,p]

Mapping: data-parallel over batch (1 image per NeuronCore, 8 cores).
BN scale folded into W1 host-side; feature map padded host-side. The 16x
broadcast of df over group channels is done for free on the TensorEngine
by replicating rows of W2 (W2exp trick): for each tap k,
Dk = W2exp_k @ x lands in PSUM already broadcast to all 256 channels.
VectorE then does acc += Dk * F_shift (one 12-row mult + add per tap).

The kernel is VectorE-bound (~2.1 ms/core predicted by TimelineSim):
fp32 tensor_tensor runs at 1x (128 lanes @ 0.96 GHz) and the involution
needs 49 taps x 256 ch x 9216 px multiply-adds = 1.8M DVE cycles floor.
PE (matmuls), ScalarE (bias+relu) and DMA all hide behind it.

Toolchain notes (hard-won):
- Must build with bacc.Bacc + nc.finalize(): Bacc.compile() splits
  semaphore waits to the 1-wait-per-instruction HW limit; raw bass.Bass
  dies in walrus with "Too many sync wait commands".
- Big preload DMAs go on gpsimd (SWDGE, one queue sem each); tiny
  observer matmuls make PE consume those sems one at a time so no
  Matmult ever needs two DMA-queue waits.
- PSUM budget: 6 banks dk (2 bufs x 3 banks) + 1 px + 1 obs = 8.
"""

import numpy as np

import concourse.bass as bass
import concourse.mybir as mybir
import concourse.tile as tile
from concourse import bacc, bass_utils

B, C, H, W = 8, 256, 96, 96
K7, PAD, G, GC, R = 7, 3, 16, 16, 64
HP = H + 2 * PAD          # 102
PIX = H * W               # 9216
BN_EPS = 1e-5
RBLK = 12                 # output rows per pipeline block
NBLK = H // RBLK          # 12
BLKPIX = RBLK * W         # 768
SUB = 384                 # matmul free-dim chunk (<=512, = 4 rows)
SUBROWS = SUB // W        # 4
NSUB = BLKPIX // SUB      # 2

F32 = mybir.dt.float32
TRACE = False

_CACHE = {}


def _build_nc():
    nc = bacc.Bacc(None, target_bir_lowering=False)
    fm_d = nc.dram_tensor("fm", [C, HP * HP], F32, kind="ExternalInput")
    gm_d = nc.dram_tensor("gm", [C, PIX], F32, kind="ExternalInput")
    w1_d = nc.dram_tensor("w1pt", [C, R], F32, kind="ExternalInput")
    bias_d = nc.dram_tensor("bias", [R, 1], F32, kind="ExternalInput")
    w2_d = nc.dram_tensor("w2et", [R, 49 * C], F32, kind="ExternalInput")
    out_d = nc.dram_tensor("out", [C, PIX], F32, kind="ExternalOutput")

    with tile.TileContext(nc) as tc:
        with tc.tile_pool(name="persist", bufs=1) as persist, \
             tc.tile_pool(name="gpool", bufs=2) as gpool, \
             tc.tile_pool(name="xpool", bufs=2) as xpool, \
             tc.tile_pool(name="accpool", bufs=2) as accpool, \
             tc.tile_pool(name="prodpool", bufs=4) as prodpool, \
             tc.tile_pool(name="psx", bufs=1, space="PSUM") as psx, \
             tc.tile_pool(name="psdk", bufs=2, space="PSUM") as psdk:

            fpad = [persist.tile([128, HP * HP], F32, tag=f"fpad{ct}", name=f"fpad{ct}")
                    for ct in range(2)]
            w1_sb = persist.tile([128, 2 * R], F32, tag="w1", name="w1sb")
            bias_sb = persist.tile([R, 1], F32, tag="bias", name="biassb")
            w2_sb = persist.tile([R, 49 * C], F32, tag="w2", name="w2sb")

            for ct in range(2):
                nc.gpsimd.dma_start(
                    out=fpad[ct][:],
                    in_=fm_d[ct * 128:(ct + 1) * 128, :])
            for ck in range(2):
                nc.gpsimd.dma_start(out=w1_sb[:, ck * R:(ck + 1) * R],
                                  in_=w1_d[ck * 128:(ck + 1) * 128, :])
            nc.gpsimd.dma_start(out=bias_sb[:], in_=bias_d[:])
            nc.gpsimd.dma_start(out=w2_sb[:], in_=w2_d[:])

            obs = psx.tile([1, 2], F32, tag="obs", name="obs", bufs=1)
            nc.tensor.matmul(obs[:, 0:1], w1_sb[:, 0:1], w1_sb[:, 0:1],
                             start=True, stop=True)
            nc.tensor.matmul(obs[:, 0:1], w1_sb[:, R:R + 1],
                             w1_sb[:, R:R + 1], start=True, stop=True)
            nc.tensor.matmul(obs[:, 1:2], w2_sb[:, 0:1], w2_sb[:, 0:1],
                             start=True, stop=True)
            vobs = persist.tile([128, 3], F32, tag="vobs", name="vobs")
            nc.vector.tensor_copy(vobs[:R, 0:1], bias_sb[:])
            nc.vector.tensor_copy(vobs[:, 1:2], fpad[0][:, 0:1])
            nc.vector.tensor_copy(vobs[:, 2:3], fpad[1][:, 0:1])

            fviews = [fpad[ct][:].rearrange("p (r j) -> p r j", j=HP)
                      for ct in range(2)]

            for blk in range(NBLK):
                r0 = blk * RBLK
                g_sb = [gpool.tile([128, BLKPIX], F32, tag=f"g{ct}", name=f"gsb{ct}")
                        for ct in range(2)]
                for ct in range(2):
                    nc.sync.dma_start(
                        out=g_sb[ct][:],
                        in_=gm_d[ct * 128:(ct + 1) * 128,
                                 r0 * W:(r0 + RBLK) * W])

                x_sb = xpool.tile([R, BLKPIX], F32, tag="x", name="xsb")
                for s in range(NSUB):
                    px = psx.tile([R, SUB], F32, tag="px", name="px")
                    for ck in range(2):
                        nc.tensor.matmul(
                            px[:], w1_sb[:, ck * R:(ck + 1) * R],
                            g_sb[ck][:, s * SUB:(s + 1) * SUB],
                            start=(ck == 0), stop=(ck == 1))
                    nc.scalar.activation(
                        x_sb[:, s * SUB:(s + 1) * SUB], px[:],
                        mybir.ActivationFunctionType.Relu,
                        bias=bias_sb[:], scale=1.0)

                acc = [accpool.tile([128, BLKPIX], F32, tag=f"acc{ct}", name=f"acc{ct}")
                       for ct in range(2)]

                for k in range(49):
                    di, dj = divmod(k, K7)
                    for ct in range(2):
                        # dk spans NSUB PSUM banks, one 4-row (384 elem)
                        # matmul per bank (N<=512/bank); one fat DVE
                        # mult+add per tap then covers all 12 rows
                        dk = psdk.tile([128, NSUB * 512], F32, tag="dk", name="dk")
                        for s in range(NSUB):
                            nc.tensor.matmul(
                                dk[:, s * 512:s * 512 + SUB],
                                w2_sb[:, k * C + ct * 128:k * C + ct * 128 + 128],
                                x_sb[:, s * SUB:(s + 1) * SUB],
                                start=True, stop=True)
                        dkv = dk[:].rearrange("p (s q) -> p s q", s=NSUB)[:, :, 0:SUB]                                    .rearrange("p s (r j) -> p s r j", j=W)
                        fsl = fviews[ct][:, r0 + di:r0 + di + RBLK, dj:dj + W]                             .rearrange("p (s r) j -> p s r j", s=NSUB)
                        accv = acc[ct][:].rearrange(
                            "p (s r j) -> p s r j", s=NSUB, j=W)
                        if k == 0:
                            nc.vector.tensor_tensor(
                                accv, dkv, fsl, mybir.AluOpType.mult)
                        else:
                            prod = prodpool.tile([128, BLKPIX], F32,
                                                 tag="prod", name="prod")
                            prodv = prod[:].rearrange(
                                "p (s r j) -> p s r j", s=NSUB, j=W)
                            nc.vector.tensor_tensor(
                                prodv, dkv, fsl, mybir.AluOpType.mult)
                            nc.vector.tensor_tensor(
                                acc[ct][:], acc[ct][:], prod[:],
                                mybir.AluOpType.add)

                for ct in range(2):
                    # residual
                    nc.vector.tensor_tensor(
                        acc[ct][:].rearrange("p (r j) -> p r j", j=W),
                        acc[ct][:].rearrange("p (r j) -> p r j", j=W),
                        fviews[ct][:, PAD + r0:PAD + r0 + RBLK, PAD:PAD + W],
                        mybir.AluOpType.add)
                    nc.sync.dma_start(
                        out=out_d[ct * 128:(ct + 1) * 128,
                                  r0 * W:(r0 + RBLK) * W],
                        in_=acc[ct][:])
    if not nc.is_finalized():
        nc.finalize()
    return nc


def _host_weights(W1, bn_gamma, bn_beta, bn_mean, bn_var, W2):
    inv = bn_gamma / np.sqrt(bn_var + BN_EPS)
    W1p = (W1 * inv[:, None]).astype(np.float32)          # (64, 256)
    w1pt = np.ascontiguousarray(W1p.T)                     # (256, 64)
    bias = (bn_beta - bn_mean * inv).astype(np.float32).reshape(R, 1)
    W2r = W2.reshape(G, 49, R)                             # [g, k, o]
    w2et = np.ascontiguousarray(
        np.repeat(W2r.transpose(2, 1, 0)[:, :, :, None], GC, axis=3)
        .reshape(R, 49 * C)).astype(np.float32)            # [o, k*256 + c]
    return w1pt, bias, w2et


def kernel(feature_map, guide_map, W1, bn_gamma, bn_beta, bn_mean, bn_var, W2):
    fm4 = np.asarray(feature_map, np.float32).reshape(B, C, H, W)
    fm = np.ascontiguousarray(
        np.pad(fm4, ((0, 0), (0, 0), (PAD, PAD), (PAD, PAD)))
        .reshape(B, C, HP * HP))
    gm = np.ascontiguousarray(np.asarray(guide_map, np.float32)
                              .reshape(B, C, PIX))
    w1pt, bias, w2et = _host_weights(
        np.asarray(W1, np.float32), np.asarray(bn_gamma, np.float32),
        np.asarray(bn_beta, np.float32), np.asarray(bn_mean, np.float32),
        np.asarray(bn_var, np.float32), np.asarray(W2, np.float32))

    if "nc" not in _CACHE:
        _CACHE["nc"] = _build_nc()
    nc = _CACHE["nc"]

    in_maps = [dict(fm=fm[i], gm=gm[i], w1pt=w1pt, bias=bias, w2et=w2et)
               for i in range(B)]
    _CACHE["in_maps"] = in_maps
    res = bass_utils.run_bass_kernel_spmd(
        nc, in_maps, core_ids=list(range(B)), trace=TRACE)
    _CACHE["last"] = res
    out = np.stack([r["out"] for r in res.results], axis=0)
    return out.reshape(B, C, H, W)



# revision 2
# speedup vs baseline: 1.0008x; 1.0008x over previous
"""Trainium2 Bass kernel for DCKModule (involution / dynamic per-pixel conv).

Math (per image, 1 image per core, 8 cores data-parallel over batch):
  x  = relu(W1p @ guide + bias)                  # (64, 9216)
  df = W2 @ x                                    # (784 = 16 g * 49 tap, 9216)
  out[c,r,j] = sum_k df[g(c),k,r,j] * fpad[c, r+di(k), j+dj(k)] + feature

Mapping (all fp16 data, fp32 PSUM accumulation):
- ROW-partition layout: image rows on SBUF partitions. A tap's row shift di
  is a partition offset of the padded-feature operand (DVE allows partition
  offsets; PE does not), the col shift dj is a free-dim offset.
- df is computed once per pixel (NOT broadcast x16 over group channels):
  per column j, PE computes x_col^T @ W2a^T -> df_j PSUM [96 r, 784], and
  ACT transpose-converts it into fp16 j-slab tiles laid out (k, g, j12).
  The x16 group-channel broadcast is free: the DVE mult reads df through a
  stride-0 AP dim.
- DVE does only the per-tap MULT, in fp16 with all-SBUF operands -> 2x DVE
  perf mode (0.52 ns/elem-row vs 1.04 fp32).
- The tap ACCUMULATION runs on the otherwise-idle PE as identity matmuls
  accumulating in PSUM fp32 (start at tap 0, stop at tap 48).
- The residual (+feature) is folded into df: x gets a constant-1 row 64 and
  W2a^T gets a row that adds 1.0 to every group's center tap (k=24).
- Output is DMA'd straight from PSUM in production order [96 r, js, ch,
  c128, j12]; the host unshuffles to (256, 96, 96). All transposes/pads of
  inputs likewise happen host-side for free.

Predicted engine busy per core (TimelineSim model): DVE ~675us (bottleneck),
PE ~570us, ACT ~100us, DMA ~55us.
"""

import numpy as np

import concourse.bass as bass
import concourse.mybir as mybir
import concourse.tile as tile
from concourse import bacc, bass_utils

B, C, H, W = 8, 256, 96, 96
K7, PAD, G, GC, R = 7, 3, 16, 16, 64
HP = H + 2 * PAD          # 102
PIX = H * W               # 9216
BN_EPS = 1e-5
JS = 12                   # j-slab width (output cols per slab)
NJS = W // JS             # 8
CH = 2                    # channel halves (128 each)
CHW = C // CH             # 128
GH = G // CH              # 8 groups per half
CENTER = PAD * K7 + PAD   # 24

F32 = mybir.dt.float32
F16 = mybir.dt.float16
TRACE = False

_CACHE = {}


FW = JS + K7 - 1          # 18: slab cols incl dj halo
FCH = K7 * CHW * FW       # 16128: one (js, ch) fpad chunk per partition


def _build_nc():
    nc = bacc.Bacc(None, target_bir_lowering=False)
    # 7 row-shifted copies of padded feature, chunked (js, ch, di, c, j):
    # DVE/PE operands must start at partition 0/32/64/96, so the tap row
    # shift di is materialized host-side instead of via partition offsets
    fpad_d = nc.dram_tensor("fpd7", [H, NJS * CH * FCH], F16,
                            kind="ExternalInput")
    gm_d = nc.dram_tensor("gm", [C, PIX], F16, kind="ExternalInput")
    w1_d = nc.dram_tensor("w1t", [C, R], F16, kind="ExternalInput")
    bias_d = nc.dram_tensor("bias", [R, 1], F32, kind="ExternalInput")
    w2_d = nc.dram_tensor("w2ta", [R + 1, G * K7 * K7], F16,
                          kind="ExternalInput")
    i96_d = nc.dram_tensor("i96", [H, H], F16, kind="ExternalInput")
    out_d = nc.dram_tensor("out", [H, C * W], F32, kind="ExternalOutput")

    NO = G * K7 * K7      # 784
    with tile.TileContext(nc) as tc:
        with tc.tile_pool(name="persist", bufs=1) as persist, \
             tc.tile_pool(name="dfpool", bufs=2) as dfpool, \
             tc.tile_pool(name="prodpool", bufs=6) as prodpool, \
             tc.tile_pool(name="pprodpool", bufs=4) as pprodpool, \
             tc.tile_pool(name="outpool", bufs=2) as outpool, \
             tc.tile_pool(name="fpool", bufs=2) as fpool, \
             tc.tile_pool(name="ps", bufs=1, space="PSUM") as ps:

            gm_sb = [persist.tile([128, PIX], F16, tag=f"gm{ct}",
                                  name=f"gm{ct}") for ct in range(2)]
            w1_sb = persist.tile([128, 2 * R], F16, tag="w1", name="w1sb")
            bias_sb = persist.tile([R, 1], F32, tag="bias", name="biassb")
            w2_sb = persist.tile([R + 1, NO], F16, tag="w2", name="w2sb")
            i96_sb = persist.tile([H, H], F16, tag="i96", name="i96sb")
            x_sb = persist.tile([R + 1, PIX], F16, tag="x", name="xsb")

            # DMA order tuned against the shared DMA device: small weights
            # first, then the gm chunks that unblock x/df(0), then fpad rows
            # 0..95 (first needed by the first tap ~22us in), then the rest
            nc.sync.dma_start(out=w1_sb[:, 0:R], in_=w1_d[0:128, :])
            nc.sync.dma_start(out=w1_sb[:, R:2 * R], in_=w1_d[128:256, :])
            nc.sync.dma_start(out=bias_sb[:], in_=bias_d[:])
            nc.sync.dma_start(out=w2_sb[:], in_=w2_d[:])

            def fetch_chunk(js, ch):
                """DMA one (js, ch) fpad chunk: [96, (di 7, c 128, j 18)]."""
                fch = fpool.tile([H, FCH], F16, tag="fch", name="fch")
                base = (js * CH + ch) * FCH
                nc.sync.dma_start(out=fch[:],
                                  in_=fpad_d[:, base:base + FCH])
                return fch[:].rearrange("p (di c j) -> p di c j",
                                        di=K7, c=CHW)

            fcur = None
            for gc_ in range(3):
                lo, hi = gc_ * 3072, (gc_ + 1) * 3072
                for ct in range(2):
                    nc.sync.dma_start(out=gm_sb[ct][:, lo:hi],
                                      in_=gm_d[ct * 128:(ct + 1) * 128,
                                               lo:hi])
                if gc_ == 0:
                    nc.sync.dma_start(out=i96_sb[:], in_=i96_d[:])
                    fcur = fetch_chunk(0, 0)

            # constant-1 row of x folds the +feature residual into df
            nc.vector.memset(x_sb[R:R + 1, :], 1.0)

            # ---- phase 1: x = relu(W1p @ guide + bias), fp16 ----
            # guide/x use j-major pixel order (pix = j*96 + r) so df columns
            # are contiguous x slices and df(0) can start after 3 x-chunks
            XCH = 512

            def x_chunk(s, borrow=False):
                if borrow:
                    px = ps.tile([H, 3 * 512], F32, tag="acc", name="px")
                else:
                    px = ps.tile([H, 1024], F32, tag="df", name="dfp")
                for ct in range(2):
                    nc.tensor.matmul(
                        px[:R, :XCH], w1_sb[:, ct * R:(ct + 1) * R],
                        gm_sb[ct][:, s * XCH:(s + 1) * XCH],
                        start=(ct == 0), stop=(ct == 1))
                nc.scalar.activation(
                    x_sb[:R, s * XCH:(s + 1) * XCH], px[:R, :XCH],
                    mybir.ActivationFunctionType.Relu, bias=bias_sb[:])

            xcols = x_sb[:].rearrange("p (j r) -> p j r", r=H)

            def alloc_slab():
                slab = dfpool.tile([H, K7 * K7 * G * JS], F16, tag="df",
                                   name="dfslab")
                sv = slab[:].rearrange("p (k g j) -> p k g j",
                                       k=K7 * K7, g=G)
                return slab, sv

            def df_column(js, sv, jl, borrow=False):
                """PE: df_j = x_col^T @ W2a^T; ACT: transpose-convert into
                the fp16 slab laid out [96 r, (k, g, j12)]."""
                j = js * JS + jl
                if borrow:
                    dfp = ps.tile([H, 3 * 512], F32, tag="acc", name="px")
                else:
                    dfp = ps.tile([H, 1024], F32, tag="df", name="dfp")
                xc = xcols[:, j, :]
                nc.tensor.matmul(dfp[:, 0:512], xc, w2_sb[:, 0:512],
                                 start=True, stop=True)
                nc.tensor.matmul(dfp[:, 512:NO], xc, w2_sb[:, 512:NO],
                                 start=True, stop=True)
                iv = dfp[:, :NO].rearrange("p (g k) -> p g k", g=G)
                nc.scalar.activation(
                    sv[:, :, :, jl].transpose([0, 2, 1]), iv,
                    mybir.ActivationFunctionType.Copy)

            # prologue: just enough x to start df(0); everything else is
            # deferred and drained at tap slots so PE's in-order stream
            # never blocks the id-add pipeline
            for s in range(3):
                x_chunk(s, borrow=True)
            cur = alloc_slab()
            for jl in range(JS):
                # alternate PSUM tags to double-buffer df(0)'s PE<->ACT chain
                df_column(0, cur[1], jl, borrow=(jl % 2 == 1))

            work = [("x", 3), ("x", 4)]
            nxt = None

            for js in range(NJS):
                # df for slab js+1 is produced column-by-column interleaved
                # into the tap loop so PE/ACT never serialize
                if js + 1 < NJS:
                    nxt = alloc_slab()
                    work.extend(("df", js + 1, nxt[1], jl)
                                for jl in range(JS))
                else:
                    nxt = None
                if js == 0:
                    work.extend(("x", s) for s in range(5, PIX // XCH))
                _, sv = cur
                for ch in range(CH):
                    # prefetch the next (js, ch) fpad chunk one phase ahead
                    nidx = js * CH + ch + 1
                    fnxt = (fetch_chunk(nidx // CH, nidx % CH)
                            if nidx < NJS * CH else None)
                    acc = ps.tile([H, CHW * JS], F32, tag="acc", name="acc")
                    for k in range(K7 * K7):
                        di, dj = divmod(k, K7)
                        # spread ~10/49 tap mults onto the idle Pool engine
                        on_pool = (k % 5 == 2)
                        if on_pool:
                            prod = pprodpool.tile([H, CHW * JS], F16,
                                                  tag="pprod", name="pprod")
                        else:
                            prod = prodpool.tile([H, CHW * JS], F16,
                                                 tag="prod", name="prod")
                        in0 = fcur[:, di, :, dj:dj + JS]
                        in1 = sv[:, k, ch * GH:(ch + 1) * GH, :] \
                            .unsqueeze(2).broadcast_to((H, GH, GC, JS))
                        pv = prod[:].rearrange("p (c j) -> p c j", j=JS)
                        eng = nc.gpsimd if on_pool else nc.vector
                        eng.tensor_tensor(pv, in0, in1,
                                          mybir.AluOpType.mult)
                        for s in range(CHW * JS // 512):
                            nc.tensor.matmul(
                                acc[:, s * 512:(s + 1) * 512], i96_sb[:],
                                prod[:, s * 512:(s + 1) * 512],
                                start=(k == 0), stop=(k == K7 * K7 - 1))
                        if work and k % 2 == 1:
                            item = work.pop(0)
                            if item[0] == "x":
                                x_chunk(item[1])
                            else:
                                df_column(item[1], item[2], item[3])
                    ev = outpool.tile([H, CHW * JS], F32, tag="ev",
                                      name="ev")
                    nc.scalar.activation(ev[:], acc[:],
                                         mybir.ActivationFunctionType.Copy)
                    nc.sync.dma_start(
                        out=out_d[:, (js * CH + ch) * CHW * JS:
                                  (js * CH + ch + 1) * CHW * JS],
                        in_=ev[:])
                    fcur = fnxt
                cur = nxt
    if not nc.is_finalized():
        nc.finalize()
    return nc


def _host_weights(W1, bn_gamma, bn_beta, bn_mean, bn_var, W2):
    inv = bn_gamma / np.sqrt(bn_var + BN_EPS)
    w1t = np.ascontiguousarray((W1 * inv[:, None]).T).astype(np.float16)
    bias = (bn_beta - bn_mean * inv).astype(np.float32).reshape(R, 1)
    w2ta = np.zeros((R + 1, G * K7 * K7), np.float16)
    w2ta[:R] = W2.T.astype(np.float16)
    w2ta[R, CENTER::K7 * K7] = 1.0
    i96 = np.eye(H, dtype=np.float16)
    return w1t, bias, w2ta, i96


def _host_fpad7(fm4):
    """[b, 96 r, (js 8, ch 2, di 7, c 128, j 18)] row-shifted fpad chunks."""
    fpad = np.pad(fm4, ((0, 0), (0, 0), (PAD, PAD), (PAD, PAD)))
    fpt = fpad.transpose(0, 2, 1, 3).astype(np.float16)  # [b, 102, 256, 102]
    out = np.empty((B, H, NJS * CH * FCH), np.float16)
    pos = 0
    for js in range(NJS):
        for ch in range(CH):
            for di in range(K7):
                blk = fpt[:, di:di + H, ch * CHW:(ch + 1) * CHW,
                          js * JS:js * JS + FW]
                out[:, :, pos:pos + CHW * FW] = blk.reshape(B, H, -1)
                pos += CHW * FW
    return out


def kernel(feature_map, guide_map, W1, bn_gamma, bn_beta, bn_mean, bn_var, W2):
    fm4 = np.asarray(feature_map, np.float32).reshape(B, C, H, W)
    fpd7 = _host_fpad7(fm4)
    gm = np.ascontiguousarray(
        np.asarray(guide_map, np.float32).reshape(B, C, H, W)
        .transpose(0, 1, 3, 2)).reshape(B, C, PIX).astype(np.float16)
    w1t, bias, w2ta, i96 = _host_weights(
        np.asarray(W1, np.float32), np.asarray(bn_gamma, np.float32),
        np.asarray(bn_beta, np.float32), np.asarray(bn_mean, np.float32),
        np.asarray(bn_var, np.float32), np.asarray(W2, np.float32))

    if "nc" not in _CACHE:
        _CACHE["nc"] = _build_nc()
    nc = _CACHE["nc"]

    in_maps = [dict(fpd7=fpd7[i], gm=gm[i], w1t=w1t, bias=bias,
                    w2ta=w2ta, i96=i96) for i in range(B)]
    _CACHE["in_maps"] = in_maps
    res = bass_utils.run_bass_kernel_spmd(
        nc, in_maps, core_ids=list(range(B)), trace=TRACE)
    _CACHE["last"] = res
    out = np.stack([r["out"] for r in res.results], axis=0)
    # [b, 96 r, (js 8, ch 2, c 128, j 12)] -> (b, 256, 96, 96)
    out = out.reshape(B, H, NJS, CH, CHW, JS).transpose(0, 3, 4, 1, 2, 5)
    return np.ascontiguousarray(out.reshape(B, C, H, W))


# revision 3
# speedup vs baseline: 1.0020x; 1.0012x over previous
"""Trainium2 Bass kernel for DCKModule (involution / dynamic per-pixel conv).

Math (per image, 1 image per core, 8 cores data-parallel over batch):
  x  = relu(W1p @ guide + bias)                  # (64, 9216)
  df = W2 @ x                                    # (784 = 16 g * 49 tap, 9216)
  out[c,r,j] = sum_k df[g(c),k,r,j] * fpad[c, r+di(k), j+dj(k)] + feature

Mapping (all fp16 data, fp32 PSUM accumulation):
- ROW-partition layout: image rows on SBUF partitions. A tap's row shift di
  is a partition offset of the padded-feature operand (DVE allows partition
  offsets; PE does not), the col shift dj is a free-dim offset.
- df is computed once per pixel (NOT broadcast x16 over group channels):
  per column j, PE computes x_col^T @ W2a^T -> df_j PSUM [96 r, 784], and
  ACT transpose-converts it into fp16 j-slab tiles laid out (k, g, j12).
  The x16 group-channel broadcast is free: the DVE mult reads df through a
  stride-0 AP dim.
- DVE does only the per-tap MULT, in fp16 with all-SBUF operands -> 2x DVE
  perf mode (0.52 ns/elem-row vs 1.04 fp32).
- The tap ACCUMULATION runs on the otherwise-idle PE as identity matmuls
  accumulating in PSUM fp32 (start at tap 0, stop at tap 48).
- The residual (+feature) is folded into df: x gets a constant-1 row 64 and
  W2a^T gets a row that adds 1.0 to every group's center tap (k=24).
- Output is DMA'd straight from PSUM in production order [96 r, js, ch,
  c128, j12]; the host unshuffles to (256, 96, 96). All transposes/pads of
  inputs likewise happen host-side for free.

Predicted engine busy per core (TimelineSim model): DVE ~675us (bottleneck),
PE ~570us, ACT ~100us, DMA ~55us.
"""

import numpy as np

import concourse.bass as bass
import concourse.mybir as mybir
import concourse.tile as tile
from concourse import bacc, bass_utils

B, C, H, W = 8, 256, 96, 96
K7, PAD, G, GC, R = 7, 3, 16, 16, 64
HP = H + 2 * PAD          # 102
PIX = H * W               # 9216
BN_EPS = 1e-5
JS = 12                   # j-slab width (output cols per slab)
NJS = W // JS             # 8
CH = 2                    # channel halves (128 each)
CHW = C // CH             # 128
GH = G // CH              # 8 groups per half
CENTER = PAD * K7 + PAD   # 24

F32 = mybir.dt.float32
F16 = mybir.dt.float16
TRACE = False

_CACHE = {}


FW = JS + K7 - 1          # 18: slab cols incl dj halo
FCH = K7 * CHW * FW       # 16128: one (js, ch) fpad chunk per partition


def _build_nc():
    nc = bacc.Bacc(None, target_bir_lowering=False)
    # 7 row-shifted copies of padded feature, chunked (js, ch, di, c, j):
    # DVE/PE operands must start at partition 0/32/64/96, so the tap row
    # shift di is materialized host-side instead of via partition offsets
    fpad_d = nc.dram_tensor("fpd7", [H, NJS * CH * FCH], F16,
                            kind="ExternalInput")
    gm_d = nc.dram_tensor("gm", [C, PIX], F16, kind="ExternalInput")
    w1_d = nc.dram_tensor("w1t", [C, R], F16, kind="ExternalInput")
    bias_d = nc.dram_tensor("bias", [R, 1], F32, kind="ExternalInput")
    w2_d = nc.dram_tensor("w2ta", [R + 1, G * K7 * K7], F16,
                          kind="ExternalInput")
    i96_d = nc.dram_tensor("i96", [H, H], F16, kind="ExternalInput")
    out_d = nc.dram_tensor("out", [H, C * W], F32, kind="ExternalOutput")

    NO = G * K7 * K7      # 784
    with tile.TileContext(nc) as tc:
        with tc.tile_pool(name="persist", bufs=1) as persist, \
             tc.tile_pool(name="dfpool", bufs=2) as dfpool, \
             tc.tile_pool(name="prodpool", bufs=8) as prodpool, \
             tc.tile_pool(name="pprodpool", bufs=3) as pprodpool, \
             tc.tile_pool(name="outpool", bufs=2) as outpool, \
             tc.tile_pool(name="fpool", bufs=2) as fpool, \
             tc.tile_pool(name="ps", bufs=1, space="PSUM") as ps:

            gm_sb = [persist.tile([128, PIX], F16, tag=f"gm{ct}",
                                  name=f"gm{ct}") for ct in range(2)]
            w1_sb = persist.tile([128, 2 * R], F16, tag="w1", name="w1sb")
            bias_sb = persist.tile([R, 1], F32, tag="bias", name="biassb")
            w2_sb = persist.tile([R + 1, NO], F16, tag="w2", name="w2sb")
            i96_sb = persist.tile([H, H], F16, tag="i96", name="i96sb")
            x_sb = persist.tile([R + 1, PIX], F16, tag="x", name="xsb")

            # DMA order tuned against the shared DMA device: small weights
            # first, then the gm chunks that unblock x/df(0), then fpad rows
            # 0..95 (first needed by the first tap ~22us in), then the rest
            nc.sync.dma_start(out=w1_sb[:, 0:R], in_=w1_d[0:128, :])
            nc.sync.dma_start(out=w1_sb[:, R:2 * R], in_=w1_d[128:256, :])
            nc.sync.dma_start(out=bias_sb[:], in_=bias_d[:])
            nc.sync.dma_start(out=w2_sb[:], in_=w2_d[:])

            def fetch_chunk(js, ch):
                """DMA one (js, ch) fpad chunk: [96, (di 7, c 128, j 18)]."""
                fch = fpool.tile([H, FCH], F16, tag="fch", name="fch")
                base = (js * CH + ch) * FCH
                nc.sync.dma_start(out=fch[:],
                                  in_=fpad_d[:, base:base + FCH])
                return fch[:].rearrange("p (di c j) -> p di c j",
                                        di=K7, c=CHW)

            fcur = None
            bounds = [0, 1536, 3072, 6144, PIX]
            for gc_ in range(4):
                lo, hi = bounds[gc_], bounds[gc_ + 1]
                for ct in range(2):
                    nc.sync.dma_start(out=gm_sb[ct][:, lo:hi],
                                      in_=gm_d[ct * 128:(ct + 1) * 128,
                                               lo:hi])
                if gc_ == 0:
                    nc.sync.dma_start(out=i96_sb[:], in_=i96_d[:])
                elif gc_ == 1:
                    fcur = fetch_chunk(0, 0)

            # constant-1 row of x folds the +feature residual into df
            nc.vector.memset(x_sb[R:R + 1, :], 1.0)

            # ---- phase 1: x = relu(W1p @ guide + bias), fp16 ----
            # guide/x use j-major pixel order (pix = j*96 + r) so df columns
            # are contiguous x slices and df(0) can start after 3 x-chunks
            XCH = 512

            def x_chunk(s, borrow=False):
                if borrow:
                    px = ps.tile([H, 3 * 512], F32, tag="acc", name="px")
                else:
                    px = ps.tile([H, 1024], F32, tag="df", name="dfp")
                for ct in range(2):
                    nc.tensor.matmul(
                        px[:R, :XCH], w1_sb[:, ct * R:(ct + 1) * R],
                        gm_sb[ct][:, s * XCH:(s + 1) * XCH],
                        start=(ct == 0), stop=(ct == 1))
                nc.scalar.activation(
                    x_sb[:R, s * XCH:(s + 1) * XCH], px[:R, :XCH],
                    mybir.ActivationFunctionType.Relu, bias=bias_sb[:])

            xcols = x_sb[:].rearrange("p (j r) -> p j r", r=H)

            def alloc_slab():
                slab = dfpool.tile([H, K7 * K7 * G * JS], F16, tag="df",
                                   name="dfslab")
                sv = slab[:].rearrange("p (k g j) -> p k g j",
                                       k=K7 * K7, g=G)
                return slab, sv

            def df_column(js, sv, jl, borrow=False):
                """PE: df_j = x_col^T @ W2a^T; ACT: transpose-convert into
                the fp16 slab laid out [96 r, (k, g, j12)]."""
                j = js * JS + jl
                if borrow:
                    dfp = ps.tile([H, 3 * 512], F32, tag="acc", name="px")
                else:
                    dfp = ps.tile([H, 1024], F32, tag="df", name="dfp")
                xc = xcols[:, j, :]
                nc.tensor.matmul(dfp[:, 0:512], xc, w2_sb[:, 0:512],
                                 start=True, stop=True)
                nc.tensor.matmul(dfp[:, 512:NO], xc, w2_sb[:, 512:NO],
                                 start=True, stop=True)
                iv = dfp[:, :NO].rearrange("p (g k) -> p g k", g=G)
                nc.scalar.activation(
                    sv[:, :, :, jl].transpose([0, 2, 1]), iv,
                    mybir.ActivationFunctionType.Copy)

            # prologue: just enough x to start df(0); everything else is
            # deferred and drained at tap slots so PE's in-order stream
            # never blocks the id-add pipeline. df(0) columns start as soon
            # as the x chunk covering them lands; PSUM tags alternate to
            # double-buffer the PE<->ACT chain.
            cur = alloc_slab()
            x_chunk(0, borrow=True)
            for jl in range(5):
                df_column(0, cur[1], jl, borrow=(jl % 2 == 1))
            x_chunk(1, borrow=True)
            for jl in range(5, 10):
                df_column(0, cur[1], jl, borrow=(jl % 2 == 1))
            x_chunk(2, borrow=True)
            for jl in range(10, JS):
                df_column(0, cur[1], jl, borrow=(jl % 2 == 1))

            work = [("x", 3), ("x", 4)]
            nxt = None

            for js in range(NJS):
                # df for slab js+1 is produced column-by-column interleaved
                # into the tap loop so PE/ACT never serialize
                if js + 1 < NJS:
                    nxt = alloc_slab()
                    work.extend(("df", js + 1, nxt[1], jl)
                                for jl in range(JS))
                else:
                    nxt = None
                if js == 0:
                    work.extend(("x", s) for s in range(5, PIX // XCH))
                _, sv = cur
                for ch in range(CH):
                    # prefetch the next (js, ch) fpad chunk one phase ahead
                    nidx = js * CH + ch + 1
                    fnxt = (fetch_chunk(nidx // CH, nidx % CH)
                            if nidx < NJS * CH else None)
                    acc = ps.tile([H, CHW * JS], F32, tag="acc", name="acc")
                    for k in range(K7 * K7):
                        di, dj = divmod(k, K7)
                        # spread ~10/49 tap mults onto the idle Pool engine;
                        # none in the first taps so PE's chain starts hot
                        on_pool = (k % 5 == 4 or k == 47)
                        if on_pool:
                            prod = pprodpool.tile([H, CHW * JS], F16,
                                                  tag="pprod", name="pprod")
                        else:
                            prod = prodpool.tile([H, CHW * JS], F16,
                                                 tag="prod", name="prod")
                        in0 = fcur[:, di, :, dj:dj + JS]
                        in1 = sv[:, k, ch * GH:(ch + 1) * GH, :] \
                            .unsqueeze(2).broadcast_to((H, GH, GC, JS))
                        pv = prod[:].rearrange("p (c j) -> p c j", j=JS)
                        eng = nc.gpsimd if on_pool else nc.vector
                        eng.tensor_tensor(pv, in0, in1,
                                          mybir.AluOpType.mult)
                        for s in range(CHW * JS // 512):
                            nc.tensor.matmul(
                                acc[:, s * 512:(s + 1) * 512], i96_sb[:],
                                prod[:, s * 512:(s + 1) * 512],
                                start=(k == 0), stop=(k == K7 * K7 - 1))
                        if work and k % 4 == 1:
                            item = work.pop(0)
                            if item[0] == "x":
                                x_chunk(item[1])
                            else:
                                df_column(item[1], item[2], item[3])
                    ev = outpool.tile([H, CHW * JS], F32, tag="ev",
                                      name="ev")
                    nc.scalar.activation(ev[:], acc[:],
                                         mybir.ActivationFunctionType.Copy)
                    nc.sync.dma_start(
                        out=out_d[:, (js * CH + ch) * CHW * JS:
                                  (js * CH + ch + 1) * CHW * JS],
                        in_=ev[:])
                    fcur = fnxt
                cur = nxt
    if not nc.is_finalized():
        nc.finalize()
    return nc


def _host_weights(W1, bn_gamma, bn_beta, bn_mean, bn_var, W2):
    inv = bn_gamma / np.sqrt(bn_var + BN_EPS)
    w1t = np.ascontiguousarray((W1 * inv[:, None]).T).astype(np.float16)
    bias = (bn_beta - bn_mean * inv).astype(np.float32).reshape(R, 1)
    w2ta = np.zeros((R + 1, G * K7 * K7), np.float16)
    w2ta[:R] = W2.T.astype(np.float16)
    w2ta[R, CENTER::K7 * K7] = 1.0
    i96 = np.eye(H, dtype=np.float16)
    return w1t, bias, w2ta, i96


def _host_fpad7(fm4):
    """[b, 96 r, (js 8, ch 2, di 7, c 128, j 18)] row-shifted fpad chunks."""
    fpad = np.pad(fm4, ((0, 0), (0, 0), (PAD, PAD), (PAD, PAD)))
    fpt = fpad.transpose(0, 2, 1, 3).astype(np.float16)  # [b, 102, 256, 102]
    out = np.empty((B, H, NJS * CH * FCH), np.float16)
    pos = 0
    for js in range(NJS):
        for ch in range(CH):
            for di in range(K7):
                blk = fpt[:, di:di + H, ch * CHW:(ch + 1) * CHW,
                          js * JS:js * JS + FW]
                out[:, :, pos:pos + CHW * FW] = blk.reshape(B, H, -1)
                pos += CHW * FW
    return out


def kernel(feature_map, guide_map, W1, bn_gamma, bn_beta, bn_mean, bn_var, W2):
    fm4 = np.asarray(feature_map, np.float32).reshape(B, C, H, W)
    fpd7 = _host_fpad7(fm4)
    gm = np.ascontiguousarray(
        np.asarray(guide_map, np.float32).reshape(B, C, H, W)
        .transpose(0, 1, 3, 2)).reshape(B, C, PIX).astype(np.float16)
    w1t, bias, w2ta, i96 = _host_weights(
        np.asarray(W1, np.float32), np.asarray(bn_gamma, np.float32),
        np.asarray(bn_beta, np.float32), np.asarray(bn_mean, np.float32),
        np.asarray(bn_var, np.float32), np.asarray(W2, np.float32))

    if "nc" not in _CACHE:
        _CACHE["nc"] = _build_nc()
    nc = _CACHE["nc"]

    in_maps = [dict(fpd7=fpd7[i], gm=gm[i], w1t=w1t, bias=bias,
                    w2ta=w2ta, i96=i96) for i in range(B)]
    _CACHE["in_maps"] = in_maps
    res = bass_utils.run_bass_kernel_spmd(
        nc, in_maps, core_ids=list(range(B)), trace=TRACE)
    _CACHE["last"] = res
    out = np.stack([r["out"] for r in res.results], axis=0)
    # [b, 96 r, (js 8, ch 2, c 128, j 12)] -> (b, 256, 96, 96)
    out = out.reshape(B, H, NJS, CH, CHW, JS).transpose(0, 3, 4, 1, 2, 5)
    return np.ascontiguousarray(out.reshape(B, C, H, W))
